# revision 1
# baseline (speedup 1.0000x reference)
"""Chamfer distance kernel for Trainium2 (8 NeuronCores, bass/tile).

Problem: X [8, 8192, 3], Y [8, 8192, 3] fp32.
  out[b] = mean_n min_m ||x_n - y_m||^2 + mean_m min_n ||x_n - y_m||^2

Strategy:
  - Data parallel over batch: core b handles batch b.
  - Distance matrix W[n,m] = |x_n|^2 + |y_m|^2 - 2 x.y is produced directly by
    the PE array as a single K=24 matmul per tile: the contraction dimension
    carries an error-free triple-bf16 splitting of X, -2Y, |x|^2, |y|^2 and
    ones, so PSUM tiles hold fp32-accurate distances at bf16 streaming speed
    (1 cycle/row vs 4 for native fp32 matmul).
  - ScalarE (ACT) casts each PSUM tile to fp16 in SBUF (the only engine with
    spare elementwise throughput; it cannot do min).
  - VectorE (DVE) does both min paths in fp16 at 2x_1P rate:
      row path: pairwise-min fold of the four 2048-wide supertiles of each
                n-tile, then a free-axis min-reduce -> rminv[:, i]
      col path: running elementwise min into a persistent [128, 8192]
                accumulator.
  - Column mins need a partition-axis reduce: PE-transpose 128x128 chunks
    (after an ACT cast back to fp32) and free-axis min-reduce each.
  - Host: means of the returned 2*8192 mins per batch.
"""

import os
import sys

sys.path.insert(0, "/opt/trn_rl_repo")

import numpy as np

B, N, M, D = 8, 8192, 8192, 3
KROWS = 24
SUPER = 2048  # psum supertile free size (4 banks)
FILL = 30000.0  # > any squared distance (~80), well below fp16 max

_CACHE = {}


def _split3_bf16(v):
    """Error-free-ish triple bf16 split: v ~= s0+s1+s2 to ~26 mantissa bits."""
    import ml_dtypes

    bf = ml_dtypes.bfloat16
    v = v.astype(np.float64)
    s0 = v.astype(bf)
    r1 = v - s0.astype(np.float64)
    s1 = r1.astype(bf)
    r2 = r1 - s1.astype(np.float64)
    s2 = r2.astype(bf)
    return s0, s1, s2


def _augment(X, Y):
    """Build [B, 24, N] bf16 lhsT rows and [B, 24, M] rhs rows such that
    sum_k XAT[k,n] * YAT[k,m] = |x_n|^2 + |y_m|^2 - 2 x_n.y_m  (fp32-accurate).
    """
    import ml_dtypes

    bf = ml_dtypes.bfloat16
    Xf = np.asarray(X, np.float64)
    Yf = np.asarray(Y, np.float64)
    X2 = (Xf * Xf).sum(-1)  # [B, N]
    Y2 = (Yf * Yf).sum(-1)  # [B, M]
    xs = _split3_bf16(np.moveaxis(Xf, -1, 1))  # 3 x [B, D, N]
    ys = _split3_bf16(np.moveaxis(-2.0 * Yf, -1, 1))  # 3 x [B, D, M]
    a = _split3_bf16(X2)  # 3 x [B, N]
    b = _split3_bf16(Y2)  # 3 x [B, M]

    nb, mb = X.shape[1], Y.shape[1]
    XAT = np.zeros((B, KROWS, nb), bf)
    YAT = np.zeros((B, KROWS, mb), bf)
    # cross terms: pairings (i,j) with i+j <= 2 capture products to ~2^-26
    pairs = [(0, 0), (0, 1), (1, 0), (0, 2), (1, 1), (2, 0)]
    r = 0
    for d in range(D):
        for (i, j) in pairs:
            XAT[:, r, :] = xs[i][:, d, :]
            YAT[:, r, :] = ys[j][:, d, :]
            r += 1
    for i in range(3):  # |x|^2 splits vs ones
        XAT[:, r, :] = a[i]
        YAT[:, r, :] = np.ones((B, mb), bf)
        r += 1
    for i in range(3):  # ones vs |y|^2 splits
        XAT[:, r, :] = np.ones((B, nb), bf)
        YAT[:, r, :] = b[i]
        r += 1
    assert r == KROWS
    return XAT, YAT


def build_module(n_rows=N, m_cols=M):
    """Build + compile the per-core bass program. Same program on all cores."""
    import concourse.bacc as bacc
    import concourse.mybir as mybir
    import concourse.tile as tile
    from concourse._compat import get_trn_type

    dt = mybir.dt
    op_min = mybir.AluOpType.min
    ax_x = mybir.AxisListType.X

    NT = n_rows // 128
    ST = m_cols // SUPER
    CT = m_cols // 128  # 128-column chunks for the transpose phase

    nc = bacc.Bacc(get_trn_type() or "TRN2", target_bir_lowering=False, debug=False)
    xat = nc.dram_tensor("xat", [KROWS, n_rows], dt.bfloat16, kind="ExternalInput")
    yat = nc.dram_tensor("yat", [KROWS, m_cols], dt.bfloat16, kind="ExternalInput")
    ident = nc.dram_tensor("ident", [128, 128], dt.float32, kind="ExternalInput")
    out = nc.dram_tensor("out", [128, NT + CT], dt.float32, kind="ExternalOutput")

    with tile.TileContext(nc) as tc:
        with (
            tc.tile_pool(name="const", bufs=1) as cpool,
            tc.tile_pool(name="acc", bufs=1) as apool,
            tc.tile_pool(name="res", bufs=1) as rpool,
        ):
            xat_sb = cpool.tile([KROWS, n_rows], dt.bfloat16)
            yat_sb = cpool.tile([KROWS, m_cols], dt.bfloat16)
            ident_sb = cpool.tile([128, 128], dt.float32)
            nc.sync.dma_start(xat_sb[:], xat[:])
            nc.sync.dma_start(yat_sb[:], yat[:])
            nc.sync.dma_start(ident_sb[:], ident[:])

            cacc = apool.tile([128, m_cols], dt.float16)
            rminv = rpool.tile([128, NT], dt.float32)
            cminv = rpool.tile([128, CT], dt.float32)
            nc.vector.memset(cacc[:], FILL)

            with (
                tc.tile_pool(name="w", bufs=8) as wpool,
                tc.tile_pool(name="rf", bufs=4) as rfpool,
                tc.tile_pool(name="ps", bufs=2, space="PSUM") as pspool,
            ):
                for i in range(NT):
                    ws = []
                    for s in range(ST):
                        ps = pspool.tile([128, SUPER], dt.float32)
                        for q in range(SUPER // 512):
                            mo = s * SUPER + q * 512
                            nc.tensor.matmul(
                                ps[:, q * 512 : (q + 1) * 512],
                                xat_sb[:, i * 128 : (i + 1) * 128],
                                yat_sb[:, mo : mo + 512],
                                start=True,
                                stop=True,
                            )
                        w = wpool.tile([128, SUPER], dt.float16, tag="w")
                        nc.scalar.copy(w[:], ps[:])
                        # col path: running min into the persistent accumulator
                        cs = cacc[:, s * SUPER : (s + 1) * SUPER]
                        nc.vector.tensor_tensor(cs, cs, w[:], op_min)
                        ws.append(w)
                    # row path: pairwise-min fold of this n-tile's supertiles
                    while len(ws) > 1:
                        nxt = []
                        for j in range(0, len(ws) - 1, 2):
                            f = rfpool.tile([128, SUPER], dt.float16, tag="rf")
                            nc.vector.tensor_tensor(f[:], ws[j][:], ws[j + 1][:], op_min)
                            nxt.append(f)
                        if len(ws) % 2:
                            nxt.append(ws[-1])
                        ws = nxt
                    last = ws[0]
                    width = SUPER
                    while width > 512:
                        h = width // 2
                        nc.vector.tensor_tensor(
                            last[:, 0:h], last[:, 0:h], last[:, h:width], op_min
                        )
                        width = h
                    nc.vector.tensor_reduce(
                        rminv[:, i : i + 1], last[:, 0:width], axis=ax_x, op=op_min
                    )

            # col path finalization: partition-axis min via PE transpose
            with (
                tc.tile_pool(name="c32", bufs=2) as c32pool,
                tc.tile_pool(name="pst", bufs=4, space="PSUM") as ptpool,
            ):
                for g in range(m_cols // SUPER):
                    c32 = c32pool.tile([128, SUPER], dt.float32)
                    nc.scalar.copy(c32[:], cacc[:, g * SUPER : (g + 1) * SUPER])
                    for c in range(SUPER // 128):
                        pt = ptpool.tile([128, 128], dt.float32)
                        nc.tensor.transpose(
                            pt[:], c32[:, c * 128 : (c + 1) * 128], ident_sb[:]
                        )
                        ci = g * (SUPER // 128) + c
                        nc.vector.tensor_reduce(
                            cminv[:, ci : ci + 1], pt[:], axis=ax_x, op=op_min
                        )

            nc.sync.dma_start(out[:, :NT], rminv[:])
            nc.sync.dma_start(out[:, NT:], cminv[:])

    nc.compile()
    return nc


def _get_module():
    if "nc" not in _CACHE:
        _CACHE["nc"] = build_module()
    return _CACHE["nc"]


def kernel(X, Y):
    from concourse import bass_utils

    X = np.asarray(X)
    Y = np.asarray(Y)
    assert X.shape == (B, N, D) and Y.shape == (B, M, D)

    XAT, YAT = _augment(X, Y)
    ident = np.eye(128, dtype=np.float32)

    nc = _get_module()
    in_maps = [
        {"xat": XAT[b], "yat": YAT[b], "ident": ident} for b in range(B)
    ]
    trace = bool(int(os.environ.get("CHAMFER_TRACE", "0")))
    r = bass_utils.run_bass_kernel_spmd(
        nc, in_maps, core_ids=list(range(B)), trace=trace
    )
    _CACHE["last_results"] = r

    NT = N // 128
    outv = np.empty((B,), np.float32)
    for b in range(B):
        o = r.results[b]["out"]  # [128, NT + CT] fp32
        rmin = o[:, :NT].astype(np.float64)
        cmin = o[:, NT:].astype(np.float64)
        outv[b] = np.float32(rmin.mean() + cmin.mean())
    return outv


# revision 21
# speedup vs baseline: 741.2472x; 741.2472x over previous
"""Chamfer distance kernel for Trainium2 (8 NeuronCores, bass/tile).

Problem: X [8, 8192, 3], Y [8, 8192, 3] fp32.
  out[b] = mean_n min_m ||x_n - y_m||^2 + mean_m min_n ||x_n - y_m||^2

Strategy:
  - Data parallel over batch: core b handles batch b.
  - Distance matrix W[n,m] = |x_n|^2 + |y_m|^2 - 2 x.y is produced directly by
    the PE array as a single K=24 matmul per tile: the contraction dimension
    carries an error-free triple-bf16 splitting of X, -2Y, |x|^2, |y|^2 and
    ones, so PSUM tiles hold fp32-accurate distances at bf16 streaming speed
    (1 cycle/row vs 4 for native fp32 matmul).
  - ScalarE (ACT) casts each PSUM tile to fp16 in SBUF (the only engine with
    spare elementwise throughput; it cannot do min).
  - VectorE (DVE) does both min paths in fp16 at 2x_1P rate:
      row path: pairwise-min fold of the four 2048-wide supertiles of each
                n-tile, then a free-axis min-reduce -> rminv[:, i]
      col path: running elementwise min into a persistent [128, 8192]
                accumulator.
  - Column mins need a partition-axis reduce: PE-transpose 128x128 chunks
    (after an ACT cast back to fp32) and free-axis min-reduce each.
  - Host: means of the returned 2*8192 mins per batch.
"""

import os
import sys

sys.path.insert(0, "/opt/trn_rl_repo")

import numpy as np

B, N, M, D = 8, 8192, 8192, 3
KROWS = 24
SUPER = 2048  # psum supertile free size (4 banks)
FILL = 30000.0  # > any squared distance (~80), well below fp16 max

_CACHE = {}


def _split3_bf16(v):
    """Error-free-ish triple bf16 split: v ~= s0+s1+s2 to ~26 mantissa bits."""
    import ml_dtypes

    bf = ml_dtypes.bfloat16
    v = v.astype(np.float64)
    s0 = v.astype(bf)
    r1 = v - s0.astype(np.float64)
    s1 = r1.astype(bf)
    r2 = r1 - s1.astype(np.float64)
    s2 = r2.astype(bf)
    return s0, s1, s2


def _augment(X, Y):
    """Build [B, 24, N] bf16 lhsT rows and [B, 24, M] rhs rows such that
    sum_k XAT[k,n] * YAT[k,m] = |x_n|^2 + |y_m|^2 - 2 x_n.y_m  (fp32-accurate).
    """
    import ml_dtypes

    bf = ml_dtypes.bfloat16
    Xf = np.asarray(X, np.float64)
    Yf = np.asarray(Y, np.float64)
    X2 = (Xf * Xf).sum(-1)  # [B, N]
    Y2 = (Yf * Yf).sum(-1)  # [B, M]
    xs = _split3_bf16(np.moveaxis(Xf, -1, 1))  # 3 x [B, D, N]
    ys = _split3_bf16(np.moveaxis(-2.0 * Yf, -1, 1))  # 3 x [B, D, M]
    a = _split3_bf16(X2)  # 3 x [B, N]
    b = _split3_bf16(Y2)  # 3 x [B, M]

    nb, mb = X.shape[1], Y.shape[1]
    XAT = np.zeros((B, KROWS, nb), bf)
    YAT = np.zeros((B, KROWS, mb), bf)
    # cross terms: pairings (i,j) with i+j <= 2 capture products to ~2^-26
    pairs = [(0, 0), (0, 1), (1, 0), (0, 2), (1, 1), (2, 0)]
    r = 0
    for d in range(D):
        for (i, j) in pairs:
            XAT[:, r, :] = xs[i][:, d, :]
            YAT[:, r, :] = ys[j][:, d, :]
            r += 1
    for i in range(3):  # |x|^2 splits vs ones
        XAT[:, r, :] = a[i]
        YAT[:, r, :] = np.ones((B, mb), bf)
        r += 1
    for i in range(3):  # ones vs |y|^2 splits
        XAT[:, r, :] = np.ones((B, nb), bf)
        YAT[:, r, :] = b[i]
        r += 1
    assert r == KROWS
    return XAT, YAT


def build_module(n_rows=N, m_cols=M, repeat=1, gp_slices=(), mode="full",
                 half_dt="bfloat16", rowgroups=1):
    """Build + compile the per-core bass program. Same program on all cores.

    repeat: run the main loop `repeat` times (idempotent mins) — used to
            measure device time as a wall-clock delta between repeat counts.
    gp_slices: unused (GPSIMD tensor_tensor is not legal on TRN2).
    mode: 'full' | 'mm' (matmuls + tiny cast probe) | 'mm_act' (no DVE min
          work) — engine-isolation probes for HW timing.
    half_dt: 'float16' or 'bfloat16' reduction dtype.
    """
    import concourse.bacc as bacc
    import concourse.mybir as mybir
    import concourse.tile as tile
    from concourse._compat import get_trn_type

    dt = mybir.dt
    hdt = getattr(dt, half_dt)
    op_min = mybir.AluOpType.min
    ax_x = mybir.AxisListType.X

    NT = n_rows // 128
    ST = m_cols // SUPER
    CT = m_cols // 128  # 128-column chunks for the transpose phase

    nc = bacc.Bacc(get_trn_type() or "TRN2", target_bir_lowering=False, debug=False)
    xat = nc.dram_tensor("xat", [KROWS, n_rows], dt.bfloat16, kind="ExternalInput")
    yat = nc.dram_tensor("yat", [KROWS, m_cols], dt.bfloat16, kind="ExternalInput")
    ident = nc.dram_tensor("ident", [128, 128], dt.float32, kind="ExternalInput")
    out = nc.dram_tensor("out", [128, NT + CT], dt.float32, kind="ExternalOutput")

    with tile.TileContext(nc) as tc:
        with (
            tc.tile_pool(name="const", bufs=1) as cpool,
            tc.tile_pool(name="acc", bufs=1) as apool,
            tc.tile_pool(name="res", bufs=1) as rpool,
        ):
            ident_sb = cpool.tile([128, 128], dt.float32)
            nc.sync.dma_start(ident_sb[:], ident[:])
            if rowgroups > 1:
                # replicate operands at partition offsets 0/32/64/96 so
                # matmuls can rotate PE row groups (LDWEIGHTS of group g
                # overlaps the in-flight MATMUL of group g-1)
                xat_sb = cpool.tile([128, n_rows], dt.bfloat16)
                yat_sb = cpool.tile([128, m_cols], dt.bfloat16)
                for g in range(rowgroups):
                    nc.sync.dma_start(xat_sb[32 * g : 32 * g + KROWS, :], xat[:])
                    nc.sync.dma_start(yat_sb[32 * g : 32 * g + KROWS, :], yat[:])
            else:
                xat_sb = cpool.tile([KROWS, n_rows], dt.bfloat16)
                yat_sb = cpool.tile([KROWS, m_cols], dt.bfloat16)
                nc.sync.dma_start(xat_sb[:], xat[:])
                nc.sync.dma_start(yat_sb[:], yat[:])

            cacc = apool.tile([128, m_cols], hdt)
            rminv = rpool.tile([128, NT], dt.float32)
            cminv = rpool.tile([128, CT], dt.float32)
            nc.vector.memset(cacc[:], FILL)
            if mode != "full":
                nc.vector.memset(rminv[:], 0.0)

            from contextlib import ExitStack

            GRAIN = 1024  # psum sub-tile (2 matmuls, 2 banks); 4 bufs = 8 banks
            NSUB = m_cols // GRAIN
            with (
                tc.tile_pool(name="w", bufs=3) as wpool,
                tc.tile_pool(name="rf", bufs=2) as rfpool,
                tc.tile_pool(name="ps", bufs=4, space="PSUM") as pspool,
            ):
                with ExitStack() as rep_ctx:
                    if repeat > 1:
                        # hardware loop: identical static body each iteration
                        # (mins are idempotent), used for timing measurements
                        rep_ctx.enter_context(tc.For_i(0, repeat, 1))
                    for i in range(NT):
                        # one contiguous fp16 W stripe per n-tile
                        wb = wpool.tile([128, m_cols], hdt, tag="w")
                        for sub in range(NSUB):
                            ps = pspool.tile([128, GRAIN], dt.float32)
                            for q in range(GRAIN // 512):
                                mo = sub * GRAIN + q * 512
                                if rowgroups > 1:
                                    g = (sub * (GRAIN // 512) + q) % rowgroups
                                    nc.tensor.matmul(
                                        ps[:, q * 512 : (q + 1) * 512],
                                        xat_sb[
                                            32 * g : 32 * g + KROWS,
                                            i * 128 : (i + 1) * 128,
                                        ],
                                        yat_sb[32 * g : 32 * g + KROWS, mo : mo + 512],
                                        start=True,
                                        stop=True,
                                        tile_position=(32 * g, 0),
                                    )
                                else:
                                    nc.tensor.matmul(
                                        ps[:, q * 512 : (q + 1) * 512],
                                        xat_sb[:, i * 128 : (i + 1) * 128],
                                        yat_sb[:, mo : mo + 512],
                                        start=True,
                                        stop=True,
                                    )
                            if mode == "mm":
                                # probe: consume each psum bank cheaply so no
                                # matmul is dead-code eliminated
                                for q in range(GRAIN // 512):
                                    nc.scalar.copy(
                                        wb[:, sub * 64 + q * 16 : sub * 64 + q * 16 + 16],
                                        ps[:, q * 512 : q * 512 + 16],
                                    )
                                continue
                            nc.scalar.copy(
                                wb[:, sub * GRAIN : (sub + 1) * GRAIN], ps[:]
                            )
                        if mode == "mm":
                            continue
                        if mode == "mm_act":
                            # probe: tiny DVE consumer, no real min work
                            nc.vector.tensor_tensor(
                                cacc[:, :64], cacc[:, :64], wb[:, :64], op_min
                            )
                            continue
                        # col path: running min into the persistent accumulator
                        CW = 4096  # fewer, larger DVE ops
                        for s in range(m_cols // CW if m_cols >= CW else 1):
                            cw = min(CW, m_cols)
                            cs = cacc[:, s * cw : (s + 1) * cw]
                            nc.vector.tensor_tensor(
                                cs, cs, wb[:, s * cw : (s + 1) * cw], op_min
                            )
                        # row path: fold the stripe in half repeatedly, then reduce
                        half = m_cols // 2
                        f = rfpool.tile([128, half], hdt, tag="rf")
                        nc.vector.tensor_tensor(
                            f[:], wb[:, :half], wb[:, half:], op_min
                        )
                        width = half
                        while width > 128:
                            h = width // 2
                            nc.vector.tensor_tensor(
                                f[:, 0:h], f[:, 0:h], f[:, h:width], op_min
                            )
                            width = h
                        nc.vector.tensor_reduce(
                            rminv[:, i : i + 1], f[:, 0:width], axis=ax_x, op=op_min
                        )

            # col path finalization: partition-axis min via PE transpose.
            # 4 transposed 128x128 chunks share one PSUM bank tile; a single
            # 3D-AP reduce then emits 4 column-min entries at once.
            with (
                tc.tile_pool(name="c32", bufs=2) as c32pool,
                tc.tile_pool(name="pst", bufs=4, space="PSUM") as ptpool,
            ):
                for g in range(m_cols // SUPER):
                    c32 = c32pool.tile([128, SUPER], dt.float32)
                    nc.scalar.copy(c32[:], cacc[:, g * SUPER : (g + 1) * SUPER])
                    for c4 in range(SUPER // 512):
                        pt = ptpool.tile([128, 4, 128], dt.float32)
                        for c in range(4):
                            nc.tensor.transpose(
                                pt[:, c, :],
                                c32[:, (c4 * 4 + c) * 128 : (c4 * 4 + c + 1) * 128],
                                ident_sb[:],
                            )
                        ci = g * (SUPER // 128) + c4 * 4
                        nc.vector.tensor_reduce(
                            cminv[:, ci : ci + 4], pt[:], axis=ax_x, op=op_min
                        )

            nc.sync.dma_start(out[:, :NT], rminv[:])
            nc.sync.dma_start(out[:, NT:], cminv[:])

    nc.compile()
    return nc


def _get_module():
    rep = int(os.environ.get("CHAMFER_REPEAT", "1"))
    half = os.environ.get("CHAMFER_HALF", "bfloat16")
    rg = int(os.environ.get("CHAMFER_RG", "1"))
    key = ("nc", rep, half, rg)
    if key not in _CACHE:
        _CACHE[key] = build_module(repeat=rep, half_dt=half, rowgroups=rg)
    return _CACHE[key]


def kernel(X, Y):
    from concourse import bass_utils

    X = np.asarray(X)
    Y = np.asarray(Y)
    assert X.shape == (B, N, D) and Y.shape == (B, M, D)

    XAT, YAT = _augment(X, Y)
    ident = np.eye(128, dtype=np.float32)

    nc = _get_module()
    in_maps = [
        {"xat": XAT[b], "yat": YAT[b], "ident": ident} for b in range(B)
    ]
    trace = bool(int(os.environ.get("CHAMFER_TRACE", "0")))
    r = bass_utils.run_bass_kernel_spmd(
        nc, in_maps, core_ids=list(range(B)), trace=trace
    )
    _CACHE["last_results"] = r

    NT = N // 128
    outv = np.empty((B,), np.float32)
    for b in range(B):
        o = r.results[b]["out"]  # [128, NT + CT] fp32
        rmin = o[:, :NT].astype(np.float64)
        cmin = o[:, NT:].astype(np.float64)
        outv[b] = np.float32(rmin.mean() + cmin.mean())
    return outv
